# revision 15
# baseline (speedup 1.0000x reference)
"""Causal multi-head attention (B=4, L=1024, D=1024, H=16) on 8 TRN2 NeuronCores.

Sharding: core c = 2*b + g handles batch b (0..3) and head group g (0..1,
8 heads each).  Each core computes QKV projections for its heads, causal
attention (upper-triangle blocks skipped; the mask and rel-pos bias are
folded into a host-packed MULTIPLICATIVE table exp(bias + mask)), and a
PARTIAL output projection against its 512 rows of w_out.  The two cores of
a batch return partial [D, L] bf16 outputs that the host sums (f32) and
transposes — no on-device collectives.

Layouts (no on-device transposes):
 - qT/kT live as [head_dim(64) on partitions, tok]; the scores matmul
   emits scores^T [kpos, q] directly.
 - v lives as [tok on partitions, 64] with a ones column appended, so the
   ctx matmul ctxT[d, q] = sum_k v[k, d] p[k, q] also accumulates softmax
   denominators into ctx row 64 for free.
 - softmax skips max-subtraction (scores are O(6); exp is safe):
   p = exp(s) * expbias, denominators divide the small [64, L] ctx rows.
 - denominator reciprocal on DVE (reciprocal_approx_fast) — the ACT engine
   only ever runs Exp (+Copy), so exactly one activation-table load.
 - the reciprocal row is broadcast across partitions with a K=1 matmul.

Scheduling: emission order = Tile scheduler priority.  The kernel is
software-pipelined so the PE never micro-idles (HAM clock-gate drops the
PE to 1.2 GHz after ~3.4us of low activity): v/qk projection tiles and
the output projection are woven between attention score/context matmuls,
scores for (h, j+2) are emitted before ctx(h, j), and head h+1's first
scores run while head h's softmax denominators resolve on DVE.
pt-mults split DVE (j<4) / GpSimd (j>=4); bias tables arrive as one
1.2MB DMA per head, two heads ahead.
"""

import functools

import ml_dtypes
import numpy as np

B, L, D, H = 4, 1024, 1024, 16
HD = D // H  # 64
HPC = H // 2  # heads per core group = 8
P = 128
NT = L // P  # 8 token/query/key tiles
NEG = -1e30

BF16 = ml_dtypes.bfloat16

# packed bias geometry, head-major: per head, blocks j=0..7 of [128, W_j]
_W = [L - P * j for j in range(NT)]
_OFF_J = [0] * NT
for _j in range(1, NT):
    _OFF_J[_j] = _OFF_J[_j - 1] + _W[_j - 1]
_HEAD_COLS = _OFF_J[-1] + _W[-1]  # 4608
_BIAS_COLS = HPC * _HEAD_COLS


def _regions(j):
    """q-ranges of the causal suffix [128j, L), split at the 512 psum-bank
    boundary so each matmul output stays inside one bank."""
    a = P * j
    if a < 512:
        return [(a, 512), (512, 1024)]
    return [(a, 1024)]


@functools.lru_cache(maxsize=1)
def _build():
    import concourse.mybir as mybir
    import concourse.tile as tile
    from concourse import bacc

    f32 = mybir.dt.float32
    bf16 = mybir.dt.bfloat16
    Exp = mybir.ActivationFunctionType.Exp

    nc = bacc.Bacc(None, target_bir_lowering=False, debug=False)

    # all inputs pre-packed host-side into SBUF layout ([partition, ...])
    # so every DMA reads >=2KB contiguous per partition row
    xt_d = nc.dram_tensor("xt", [P, 2 * NT * 512], bf16, kind="ExternalInput")
    wqk_d = nc.dram_tensor("wqk", [P, NT * NT * P], bf16, kind="ExternalInput")
    wv_d = nc.dram_tensor("wv", [P, NT * HPC * HD], bf16, kind="ExternalInput")
    wout_d = nc.dram_tensor("wout", [P, 4 * D], bf16, kind="ExternalInput")
    bias_d = nc.dram_tensor("biasp", [P, _BIAS_COLS], bf16, kind="ExternalInput")
    outp_d = nc.dram_tensor("outp", [D, L], bf16, kind="ExternalOutput")

    with tile.TileContext(nc) as tc:
        with (
            tc.tile_pool(name="persist", bufs=1) as keep,
            tc.tile_pool(name="bias", bufs=3) as bias_pool,
            tc.tile_pool(name="ptr", bufs=3) as ptr_pool,
            tc.tile_pool(name="pt", bufs=3) as pt_pool,
            tc.tile_pool(name="recip", bufs=2) as r_pool,
            tc.tile_pool(name="stg", bufs=2) as stg_pool,
            tc.tile_pool(name="osb", bufs=2) as out_pool,
            tc.tile_pool(name="psum", bufs=2, space="PSUM") as psum,
        ):
            # xt u-major, wqk t-major: DMA destinations are contiguous per
            # partition so transfers run at full segment size
            xt_sb = keep.tile([P, 2, NT, 512], bf16, tag="xt")
            wqk_sb = keep.tile([P, NT, NT, P], bf16, tag="wqk")
            wv_sb = keep.tile([P, NT, HPC * HD], bf16, tag="wv")
            wout_sb = keep.tile([P, HPC * HD // P, D], bf16, tag="wout")
            qt_sb = keep.tile([P, HPC // 2, L], bf16, tag="qt")
            kt_sb = keep.tile([P, HPC // 2, L], bf16, tag="kt")
            vaug_sb = keep.tile([P, NT, HPC, HD + 1], bf16, tag="vaug")
            ctx_sb = keep.tile([P, HPC * HD // P, L], bf16, tag="ctx")
            ones_sb = keep.tile([65, HD], bf16, tag="ones")

            def dma_wqk(t):
                # host layout [p, t, dt, c]: 2KB contiguous per partition
                nc.sync.dma_start(
                    wqk_sb[:, t, :, :],
                    wqk_d[:, NT * P * t : NT * P * (t + 1)].rearrange(
                        "p (dt c) -> p dt c", c=P
                    ),
                )

            def dma_xt(u, ta, tb):
                # host layout [p, u, dt, c]: 4KB contiguous per chunk
                nc.scalar.dma_start(
                    xt_sb[:, u, ta:tb, :],
                    xt_d[
                        :, NT * 512 * u + 512 * ta : NT * 512 * u + 512 * tb
                    ].rearrange("p (dt c) -> p dt c", c=512),
                )

            bias_tiles = {}

            def dma_bias(h, eng=None):
                bt = bias_pool.tile([P, _HEAD_COLS], bf16, tag="bias")
                (eng or nc.sync).dma_start(
                    bt[:], bias_d[:, h * _HEAD_COLS : (h + 1) * _HEAD_COLS]
                )
                bias_tiles[h] = bt

            # spread the prologue DMAs across idle engine queues (a direct
            # DMA occupies its issuing queue for the transfer duration):
            # sync: wqk, later bias; scalar: xt; vector: wv+wout;
            # gpsimd: first two bias tables.
            dma_wqk(0)
            dma_xt(0, 0, 4)
            dma_xt(0, 4, 8)
            dma_wqk(4)
            dma_xt(1, 0, 4)
            dma_xt(1, 4, 8)
            dma_bias(0, nc.gpsimd)
            nc.gpsimd.dma_start(
                wv_sb[:], wv_d.rearrange("p (dt c) -> p dt c", c=HPC * HD)
            )
            dma_bias(1, nc.gpsimd)
            for t in (1, 5, 2, 6, 3, 7):
                dma_wqk(t)
            nc.gpsimd.dma_start(
                wout_sb[:], wout_d.rearrange("p (dt c) -> p dt c", c=D)
            )
            nc.vector.memset(vaug_sb[:, :, :, HD : HD + 1], 1.0)
            nc.vector.memset(ones_sb[64:65, :], 1.0)

            # ---- projection tiles (emitted interleaved with attention) ----
            def qk_tile(t, cast_scalar=False):
                # col-tile t: t<4 -> q head-pair t ; t>=4 -> k head-pair t-4
                for u in range(2):
                    ps = psum.tile([P, 512], f32, tag="qkv", name=f"qkps{t}{u}")
                    for dt in range(NT):
                        nc.tensor.matmul(
                            ps[:],
                            wqk_sb[:, t, dt, :],
                            xt_sb[:, u, dt, :],
                            start=(dt == 0),
                            stop=(dt == NT - 1),
                        )
                    dest = qt_sb if t < 4 else kt_sb
                    dst = dest[:, t % 4, 512 * u : 512 * (u + 1)]
                    if cast_scalar:
                        nc.scalar.copy(dst, ps[:])
                    else:
                        nc.vector.tensor_copy(dst, ps[:])

            def v_tile(tt, cast_scalar=False):
                ps = psum.tile([P, 512], f32, tag="qkv", name=f"vps{tt}")
                for dt in range(NT):
                    nc.tensor.matmul(
                        ps[:],
                        xt_sb[:, tt // 4, dt, P * (tt % 4) : P * (tt % 4 + 1)],
                        wv_sb[:, dt, :],
                        start=(dt == 0),
                        stop=(dt == NT - 1),
                    )
                dst = vaug_sb[:, tt, :, 0:HD]
                src = ps[:].rearrange("p (h d) -> p h d", d=HD)
                if cast_scalar:
                    nc.scalar.copy(dst, src)
                else:
                    nc.vector.tensor_copy(dst, src)

            # ---- attention pieces ----
            # per-head state: sc/pt tiles in flight, ctx accumulators
            clo = {}
            chi = {}
            pt_tiles = {}

            def sco(h, j):
                """scores matmuls + exp + bias-mult for (h, j)."""
                pair, base = h // 2, 64 * (h % 2)
                a0 = P * j
                w = _W[j]
                if j < 4:
                    sc = psum.tile([P, L], f32, tag="sc", name=f"sc{h}{j}")
                    off = 0
                else:
                    # narrow: 1 bank holding global cols [512, 1024)
                    sc = psum.tile([P, 512], f32, tag="qkv", name=f"sc{h}{j}")
                    off = 512
                for a, b in _regions(j):
                    nc.tensor.matmul(
                        sc[:, a - off : b - off],
                        kt_sb[base : base + 64, pair, P * j : P * (j + 1)],
                        qt_sb[base : base + 64, pair, a:b],
                        start=True,
                        stop=True,
                    )
                ptr = ptr_pool.tile([P, L], bf16, tag="ptr")
                nc.scalar.activation(ptr[:, a0:], sc[:, a0 - off :], Exp)
                pt = pt_pool.tile([P, L], bf16, tag="pt")
                eng = nc.vector if j < 4 else nc.gpsimd
                boff = _OFF_J[j]
                eng.tensor_mul(
                    pt[:, a0:], ptr[:, a0:], bias_tiles[h][:, boff : boff + w]
                )
                pt_tiles[(h, j)] = pt

            def ctx(h, j):
                """context accumulation matmuls for (h, j)."""
                if j == 0:
                    clo[h] = psum.tile([HD + 1, 512], f32, tag="ctx", name=f"clo{h}")
                    chi[h] = psum.tile([HD + 1, 512], f32, tag="ctx", name=f"chi{h}")
                pt = pt_tiles.pop((h, j))
                for a, b in _regions(j):
                    if a < 512:
                        dst, st, sp = clo[h][:, a:512], (j == 0), (j == 3)
                    else:
                        dst, st, sp = (
                            chi[h][:, a - 512 : 512],
                            (j == 0),
                            (j == NT - 1),
                        )
                    nc.tensor.matmul(
                        dst,
                        vaug_sb[:, j, h, :],
                        pt[:, a:b],
                        start=st,
                        stop=sp,
                    )

            craw_t = {}
            denb_t = {}

            def denom_pre(h):
                """DVE chain: copy ctx+sums to SBUF (frees psum), cast the
                denominator row to bf16 for the PE broadcast."""
                craw = r_pool.tile([65, L], f32, tag="craw")
                denb = r_pool.tile([65, L], bf16, tag="denb")
                for u, cp in ((0, clo.pop(h)), (1, chi.pop(h))):
                    nc.vector.tensor_copy(craw[:, 512 * u : 512 * (u + 1)], cp[:])
                with nc.allow_low_precision(reason="bf16 denom bcast"):
                    nc.vector.tensor_copy(denb[64:65, :], craw[64:65, :])
                craw_t[h] = craw
                denb_t[h] = denb

            def bcast_norm(h):
                """K=1 broadcast matmul of the denominator row, approx
                reciprocal on the (offset-0) broadcast, then the normalizing
                mults into ctx_sb (even h) / staging (odd h)."""
                pair = h // 2
                craw, denb = craw_t.pop(h), denb_t.pop(h)
                # odd head (second of the pair, finishes last) writes direct;
                # even head stages + DMAs to partitions 64-127.  Host packs
                # w_out rows per pair as [head 2p+1, head 2p] to match.
                if h % 2 == 1:
                    dst = ctx_sb[0:64, pair, :]
                else:
                    stg = stg_pool.tile([64, L], bf16, tag="stg")
                    dst = stg[:]
                for u in range(2):
                    rb = psum.tile([P, 512], f32, tag="qkv", name=f"rbc{h}{u}")
                    with nc.allow_low_precision(reason="bf16 bcast mm"):
                        nc.tensor.matmul(
                            rb[0:64, :],
                            ones_sb[64:65, :],
                            denb[64:65, 512 * u : 512 * (u + 1)],
                            start=True,
                            stop=True,
                        )
                    rbr = r_pool.tile([64, 512], f32, tag="rbr")
                    nc.vector.reciprocal_approx_fast(out=rbr[:], in_=rb[0:64, :])
                    nc.vector.tensor_mul(
                        dst[:, 512 * u : 512 * (u + 1)],
                        craw[0:64, 512 * u : 512 * (u + 1)],
                        rbr[:],
                    )
                if h % 2 == 0:
                    nc.sync.dma_start(ctx_sb[64:128, pair, :], dst)

            # ---- emission schedule ----
            # head 0 ramps up with the v tiles woven in; the remaining qk
            # pairs are fillers at the start of heads 1..6.
            qk_tile(0, cast_scalar=True)
            qk_tile(4, cast_scalar=True)
            sco(0, 0)
            v_tile(0, cast_scalar=True)
            sco(0, 1)
            v_tile(1, cast_scalar=True)
            for j in range(6):
                ctx(0, j)
                sco(0, j + 2)
                if j < 6:
                    v_tile(j + 2, cast_scalar=(j < 2))
            ctx(0, 6)
            ctx(0, 7)

            # both qk tiles of pair p+1 run as fillers during the FIRST head
            # of pair p, so the pair-boundary scores never wait on a cast
            fillers = {1: (1, 5), 3: (2, 6), 5: (3, 7)}
            for h in range(1, HPC):
                if h + 1 < HPC:
                    dma_bias(h + 1)
                denom_pre(h - 1)
                for t in fillers.get(h, ()):
                    qk_tile(t)
                sco(h, 0)
                sco(h, 1)
                bcast_norm(h - 1)
                for j in range(6):
                    ctx(h, j)
                    sco(h, j + 2)
                ctx(h, 6)
                ctx(h, 7)
            denom_pre(HPC - 1)
            bcast_norm(HPC - 1)

            # ---- partial output projection (tail) ----
            nd = HPC * HD // P  # 4 d-tiles
            for et in range(NT):
                ot = out_pool.tile([P, L], bf16, tag="osb")
                ps = psum.tile([P, L], f32, tag="sc", name=f"ops{et}")
                for u in range(2):
                    for dt in range(nd):
                        nc.tensor.matmul(
                            ps[:, 512 * u : 512 * (u + 1)],
                            wout_sb[:, dt, P * et : P * (et + 1)],
                            ctx_sb[:, dt, 512 * u : 512 * (u + 1)],
                            start=(dt == 0),
                            stop=(dt == nd - 1),
                        )
                # casts/DMAs alternate engines/queues so the tail drains fast
                if et % 2 == 0:
                    with nc.allow_low_precision(reason="bf16 partial out"):
                        nc.vector.tensor_copy(ot[:], ps[:])
                else:
                    nc.scalar.copy(ot[:], ps[:])
                (nc.gpsimd if et % 2 else nc.sync).dma_start(
                    outp_d[P * et : P * (et + 1), :], ot[:]
                )

    nc.compile()
    return nc


def _prep_core_inputs(x, mask, w_qkv, w_out, rel_pos_bias):
    """Host-side sharding/layout prep.  Returns in_maps for the 8 cores."""
    w3 = w_qkv.reshape(D, 3, H, HD)
    madd_t = np.where(mask[0, 0], np.float32(0), np.float32(NEG)).T  # [k, q]
    scale = np.float32(HD**-0.5)

    in_maps = []
    for c in range(8):
        b, g = divmod(c, 2)
        hs = slice(g * HPC, (g + 1) * HPC)
        # xt packed [p, u, dt, c]: SBUF layout, 4KB-contiguous DMA chunks
        xt = np.ascontiguousarray(
            x[b].T.reshape(NT, P, 2, 512).transpose(1, 2, 0, 3).reshape(P, -1)
        ).astype(BF16)
        qpart = (w3[:, 0, hs, :] * scale).reshape(D, HPC * HD)
        kpart = w3[:, 1, hs, :].reshape(D, HPC * HD)
        wqk_flat = np.concatenate([qpart, kpart], axis=1)  # [D, 1024]
        # packed [p, t, dt, c]: one contiguous 2KB/partition DMA per col-tile
        wqk = np.ascontiguousarray(
            wqk_flat.reshape(NT, P, NT, P).transpose(1, 2, 0, 3).reshape(P, -1)
        ).astype(BF16)
        wv = np.ascontiguousarray(
            w3[:, 2, hs, :].reshape(NT, P, HPC * HD).transpose(1, 0, 2).reshape(P, -1)
        ).astype(BF16)
        # w_out rows: per pair, second head first (matches ctx_sb layout
        # where the odd head writes partitions 0-63 directly)
        wo = w_out[g * HPC * HD : (g + 1) * HPC * HD, :]
        wo = wo.reshape(4, 2, HD, D)[:, ::-1].reshape(HPC * HD, D)
        wout = np.ascontiguousarray(
            wo.reshape(4, P, D).transpose(1, 0, 2).reshape(P, -1)
        ).astype(BF16)

        # multiplicative bias table: exp(bias + additive mask), head-major
        biasp = np.empty((P, _BIAS_COLS), dtype=np.float32)
        bt = rel_pos_bias[hs].transpose(0, 2, 1)  # [8, k, q]
        for hl in range(HPC):
            for j in range(NT):
                blk = bt[hl, P * j : P * (j + 1), P * j : L] + madd_t[
                    P * j : P * (j + 1), P * j : L
                ]  # [128, W_j]
                o = hl * _HEAD_COLS + _OFF_J[j]
                biasp[:, o : o + _W[j]] = np.exp(blk)
        in_maps.append(
            {
                "xt": xt,
                "wqk": wqk,
                "wv": wv,
                "wout": wout,
                "biasp": biasp.astype(BF16),
            }
        )
    return in_maps


# test-harness hooks (ignored in normal grading use)
PROFILE_DIR = None
TRACE_CORES = None
LAST_RESULT = None


def kernel(x, mask, w_qkv, w_out, rel_pos_bias):
    from concourse.bass_utils import run_bass_kernel_spmd

    global LAST_RESULT
    nc = _build()
    in_maps = _prep_core_inputs(x, mask, w_qkv, w_out, rel_pos_bias)
    kwargs = {}
    if PROFILE_DIR is not None:
        kwargs = dict(
            trace=True,
            tmpdir=PROFILE_DIR,
            trace_cores=TRACE_CORES,
        )
    res = run_bass_kernel_spmd(nc, in_maps, core_ids=list(range(8)), **kwargs)
    LAST_RESULT = res
    out = np.empty((B, L, D), np.float32)
    for b in range(B):
        part = res.results[2 * b]["outp"].astype(np.float32) + res.results[
            2 * b + 1
        ]["outp"].astype(np.float32)
        out[b] = part.T
    return out


# revision 17
# speedup vs baseline: 1.0990x; 1.0990x over previous
"""Causal multi-head attention (B=4, L=1024, D=1024, H=16) on 8 TRN2 NeuronCores.

Sharding: core c = 2*b + g handles batch b (0..3) and head group g (0..1,
8 heads each).  Each core computes QKV projections for its heads, causal
attention (upper-triangle blocks skipped; the mask and rel-pos bias are
folded into a host-packed MULTIPLICATIVE table exp(bias + mask)), and a
PARTIAL output projection against its 512 rows of w_out.  The two cores of
a batch return partial [D, L] bf16 outputs that the host sums (f32) and
transposes — no on-device collectives.

Layouts (no on-device transposes):
 - qT/kT live as [head_dim(64) on partitions, tok]; the scores matmul
   emits scores^T [kpos, q] directly.
 - v lives as [tok on partitions, 64] with a ones column appended, so the
   ctx matmul ctxT[d, q] = sum_k v[k, d] p[k, q] also accumulates softmax
   denominators into ctx row 64 for free.
 - softmax skips max-subtraction (scores are O(6); exp is safe):
   p = exp(s) * expbias, denominators divide the small [64, L] ctx rows.
 - denominator reciprocal on DVE (reciprocal_approx_fast) — the ACT engine
   only ever runs Exp (+Copy), so exactly one activation-table load.
 - the reciprocal row is broadcast across partitions with a K=1 matmul.

Scheduling: emission order = Tile scheduler priority.  The kernel is
software-pipelined so the PE never micro-idles (HAM clock-gate drops the
PE to 1.2 GHz after ~3.4us of low activity): v/qk projection tiles and
the output projection are woven between attention score/context matmuls,
scores for (h, j+2) are emitted before ctx(h, j), and head h+1's first
scores run while head h's softmax denominators resolve on DVE.
pt-mults split DVE (j<4) / GpSimd (j>=4); bias tables arrive as one
1.2MB DMA per head, two heads ahead.
"""

import functools

import ml_dtypes
import numpy as np

B, L, D, H = 4, 1024, 1024, 16
HD = D // H  # 64
HPC = H // 2  # heads per core group = 8
P = 128
NT = L // P  # 8 token/query/key tiles
NEG = -1e30

BF16 = ml_dtypes.bfloat16

# packed bias geometry, head-major: per head, blocks j=0..7 of [128, W_j]
_W = [L - P * j for j in range(NT)]
_OFF_J = [0] * NT
for _j in range(1, NT):
    _OFF_J[_j] = _OFF_J[_j - 1] + _W[_j - 1]
_HEAD_COLS = _OFF_J[-1] + _W[-1]  # 4608
_BIAS_COLS = HPC * _HEAD_COLS


def _regions(j):
    """q-ranges of the causal suffix [128j, L), split at the 512 psum-bank
    boundary so each matmul output stays inside one bank."""
    a = P * j
    if a < 512:
        return [(a, 512), (512, 1024)]
    return [(a, 1024)]


@functools.lru_cache(maxsize=1)
def _build():
    import concourse.mybir as mybir
    import concourse.tile as tile
    from concourse import bacc

    f32 = mybir.dt.float32
    bf16 = mybir.dt.bfloat16
    Exp = mybir.ActivationFunctionType.Exp

    nc = bacc.Bacc(None, target_bir_lowering=False, debug=False)

    # all inputs pre-packed host-side into SBUF layout ([partition, ...])
    # so every DMA reads >=2KB contiguous per partition row
    xt_d = nc.dram_tensor("xt", [P, 2 * NT * 512], bf16, kind="ExternalInput")
    wqk_d = nc.dram_tensor("wqk", [P, NT * NT * P], bf16, kind="ExternalInput")
    wv_d = nc.dram_tensor("wv", [P, NT * HPC * HD], bf16, kind="ExternalInput")
    wout_d = nc.dram_tensor("wout", [P, 4 * D], bf16, kind="ExternalInput")
    bias_d = nc.dram_tensor("biasp", [P, _BIAS_COLS], bf16, kind="ExternalInput")
    outp_d = nc.dram_tensor("outp", [D, L], bf16, kind="ExternalOutput")

    with tile.TileContext(nc) as tc:
        with (
            tc.tile_pool(name="persist", bufs=1) as keep,
            tc.tile_pool(name="bias", bufs=3) as bias_pool,
            tc.tile_pool(name="ptr", bufs=3) as ptr_pool,
            tc.tile_pool(name="pt", bufs=3) as pt_pool,
            tc.tile_pool(name="recip", bufs=2) as r_pool,
            tc.tile_pool(name="stg", bufs=2) as stg_pool,
            tc.tile_pool(name="osb", bufs=2) as out_pool,
            tc.tile_pool(name="psum", bufs=2, space="PSUM") as psum,
        ):
            # xt u-major, wqk t-major: DMA destinations are contiguous per
            # partition so transfers run at full segment size
            xt_sb = keep.tile([P, 2, NT, 512], bf16, tag="xt")
            wqk_sb = keep.tile([P, NT, NT, P], bf16, tag="wqk")
            wv_sb = keep.tile([P, NT, HPC * HD], bf16, tag="wv")
            wout_sb = keep.tile([P, HPC * HD // P, D], bf16, tag="wout")
            qt_sb = keep.tile([P, HPC // 2, L], bf16, tag="qt")
            kt_sb = keep.tile([P, HPC // 2, L], bf16, tag="kt")
            vaug_sb = keep.tile([P, NT, HPC, HD + 1], bf16, tag="vaug")
            ctx_sb = keep.tile([P, HPC * HD // P, L], bf16, tag="ctx")
            ones_sb = keep.tile([65, HD], bf16, tag="ones")

            def dma_wqk(t):
                # host layout [p, t, dt, c]: 2KB contiguous per partition
                nc.sync.dma_start(
                    wqk_sb[:, t, :, :],
                    wqk_d[:, NT * P * t : NT * P * (t + 1)].rearrange(
                        "p (dt c) -> p dt c", c=P
                    ),
                )

            def dma_xt(u, ta, tb):
                # host layout [p, u, dt, c]: 4KB contiguous per chunk
                nc.sync.dma_start(
                    xt_sb[:, u, ta:tb, :],
                    xt_d[
                        :, NT * 512 * u + 512 * ta : NT * 512 * u + 512 * tb
                    ].rearrange("p (dt c) -> p dt c", c=512),
                )

            bias_tiles = {}

            def dma_bias(h, eng=None):
                bt = bias_pool.tile([P, _HEAD_COLS], bf16, tag="bias")
                (eng or nc.sync).dma_start(
                    bt[:], bias_d[:, h * _HEAD_COLS : (h + 1) * _HEAD_COLS]
                )
                bias_tiles[h] = bt

            # ONE prioritized queue: the prologue is HBM-bandwidth-bound, so
            # parallel queues only let non-critical transfers steal bandwidth
            # from the critical path.  Strict arrival order instead.
            dma_wqk(0)
            dma_xt(0, 0, 4)
            dma_xt(0, 4, 8)
            dma_wqk(4)
            dma_xt(1, 0, 4)
            dma_xt(1, 4, 8)
            nc.sync.dma_start(
                wv_sb[:], wv_d.rearrange("p (dt c) -> p dt c", c=HPC * HD)
            )
            dma_bias(0)
            dma_bias(1)
            for t in (1, 5, 2, 6, 3, 7):
                dma_wqk(t)
            nc.sync.dma_start(
                wout_sb[:], wout_d.rearrange("p (dt c) -> p dt c", c=D)
            )
            nc.vector.memset(vaug_sb[:, :, :, HD : HD + 1], 1.0)
            nc.vector.memset(ones_sb[64:65, :], 1.0)

            # ---- projection tiles (emitted interleaved with attention) ----
            def qk_tile(t, cast_scalar=False):
                # col-tile t: t<4 -> q head-pair t ; t>=4 -> k head-pair t-4
                for u in range(2):
                    ps = psum.tile([P, 512], f32, tag="qkv", name=f"qkps{t}{u}")
                    for dt in range(NT):
                        nc.tensor.matmul(
                            ps[:],
                            wqk_sb[:, t, dt, :],
                            xt_sb[:, u, dt, :],
                            start=(dt == 0),
                            stop=(dt == NT - 1),
                        )
                    dest = qt_sb if t < 4 else kt_sb
                    dst = dest[:, t % 4, 512 * u : 512 * (u + 1)]
                    if cast_scalar:
                        nc.scalar.copy(dst, ps[:])
                    else:
                        nc.vector.tensor_copy(dst, ps[:])

            def v_tile(tt, cast_scalar=False):
                ps = psum.tile([P, 512], f32, tag="qkv", name=f"vps{tt}")
                for dt in range(NT):
                    nc.tensor.matmul(
                        ps[:],
                        xt_sb[:, tt // 4, dt, P * (tt % 4) : P * (tt % 4 + 1)],
                        wv_sb[:, dt, :],
                        start=(dt == 0),
                        stop=(dt == NT - 1),
                    )
                dst = vaug_sb[:, tt, :, 0:HD]
                src = ps[:].rearrange("p (h d) -> p h d", d=HD)
                if cast_scalar:
                    nc.scalar.copy(dst, src)
                else:
                    nc.vector.tensor_copy(dst, src)

            # ---- attention pieces ----
            # per-head state: sc/pt tiles in flight, ctx accumulators
            clo = {}
            chi = {}
            pt_tiles = {}

            def sco(h, j):
                """scores matmuls + exp + bias-mult for (h, j)."""
                pair, base = h // 2, 64 * (h % 2)
                a0 = P * j
                w = _W[j]
                if j < 4:
                    sc = psum.tile([P, L], f32, tag="sc", name=f"sc{h}{j}")
                    off = 0
                else:
                    # narrow: 1 bank holding global cols [512, 1024)
                    sc = psum.tile([P, 512], f32, tag="qkv", name=f"sc{h}{j}")
                    off = 512
                for a, b in _regions(j):
                    nc.tensor.matmul(
                        sc[:, a - off : b - off],
                        kt_sb[base : base + 64, pair, P * j : P * (j + 1)],
                        qt_sb[base : base + 64, pair, a:b],
                        start=True,
                        stop=True,
                    )
                ptr = ptr_pool.tile([P, L], bf16, tag="ptr")
                nc.scalar.activation(ptr[:, a0:], sc[:, a0 - off :], Exp)
                pt = pt_pool.tile([P, L], bf16, tag="pt")
                eng = nc.vector if j < 4 else nc.gpsimd
                boff = _OFF_J[j]
                eng.tensor_mul(
                    pt[:, a0:], ptr[:, a0:], bias_tiles[h][:, boff : boff + w]
                )
                pt_tiles[(h, j)] = pt

            def ctx(h, j):
                """context accumulation matmuls for (h, j)."""
                if j == 0:
                    clo[h] = psum.tile([HD + 1, 512], f32, tag="ctx", name=f"clo{h}")
                    chi[h] = psum.tile([HD + 1, 512], f32, tag="ctx", name=f"chi{h}")
                pt = pt_tiles.pop((h, j))
                for a, b in _regions(j):
                    if a < 512:
                        dst, st, sp = clo[h][:, a:512], (j == 0), (j == 3)
                    else:
                        dst, st, sp = (
                            chi[h][:, a - 512 : 512],
                            (j == 0),
                            (j == NT - 1),
                        )
                    nc.tensor.matmul(
                        dst,
                        vaug_sb[:, j, h, :],
                        pt[:, a:b],
                        start=st,
                        stop=sp,
                    )

            craw_t = {}
            denb_t = {}

            def denom_pre(h):
                """DVE chain: copy ctx+sums to SBUF (frees psum), cast the
                denominator row to bf16 for the PE broadcast."""
                craw = r_pool.tile([65, L], f32, tag="craw")
                denb = r_pool.tile([65, L], bf16, tag="denb")
                for u, cp in ((0, clo.pop(h)), (1, chi.pop(h))):
                    nc.vector.tensor_copy(craw[:, 512 * u : 512 * (u + 1)], cp[:])
                with nc.allow_low_precision(reason="bf16 denom bcast"):
                    nc.vector.tensor_copy(denb[64:65, :], craw[64:65, :])
                craw_t[h] = craw
                denb_t[h] = denb

            def bcast_norm(h):
                """K=1 broadcast matmul of the denominator row, approx
                reciprocal on the (offset-0) broadcast, then the normalizing
                mults into ctx_sb (even h) / staging (odd h)."""
                pair = h // 2
                craw, denb = craw_t.pop(h), denb_t.pop(h)
                # odd head (second of the pair, finishes last) writes direct;
                # even head stages + DMAs to partitions 64-127.  Host packs
                # w_out rows per pair as [head 2p+1, head 2p] to match.
                if h % 2 == 1:
                    dst = ctx_sb[0:64, pair, :]
                else:
                    stg = stg_pool.tile([64, L], bf16, tag="stg")
                    dst = stg[:]
                for u in range(2):
                    rb = psum.tile([P, 512], f32, tag="qkv", name=f"rbc{h}{u}")
                    with nc.allow_low_precision(reason="bf16 bcast mm"):
                        nc.tensor.matmul(
                            rb[0:64, :],
                            ones_sb[64:65, :],
                            denb[64:65, 512 * u : 512 * (u + 1)],
                            start=True,
                            stop=True,
                        )
                    rbr = r_pool.tile([64, 512], f32, tag="rbr")
                    nc.vector.reciprocal_approx_fast(out=rbr[:], in_=rb[0:64, :])
                    nc.vector.tensor_mul(
                        dst[:, 512 * u : 512 * (u + 1)],
                        craw[0:64, 512 * u : 512 * (u + 1)],
                        rbr[:],
                    )
                if h % 2 == 0:
                    nc.sync.dma_start(ctx_sb[64:128, pair, :], dst)

            # ---- emission schedule ----
            # head 0 ramps up with the v tiles woven in; the remaining qk
            # pairs are fillers at the start of heads 1..6.
            qk_tile(0, cast_scalar=True)
            qk_tile(4, cast_scalar=True)
            sco(0, 0)
            v_tile(0, cast_scalar=True)
            sco(0, 1)
            v_tile(1, cast_scalar=True)
            for j in range(6):
                ctx(0, j)
                sco(0, j + 2)
                if j < 6:
                    v_tile(j + 2, cast_scalar=(j < 2))
            ctx(0, 6)
            ctx(0, 7)

            # both qk tiles of pair p+1 run as fillers during the FIRST head
            # of pair p, so the pair-boundary scores never wait on a cast
            fillers = {1: (1, 5), 3: (2, 6), 5: (3, 7)}
            for h in range(1, HPC):
                if h + 1 < HPC:
                    dma_bias(h + 1)
                denom_pre(h - 1)
                for t in fillers.get(h, ()):
                    qk_tile(t)
                sco(h, 0)
                sco(h, 1)
                bcast_norm(h - 1)
                for j in range(6):
                    ctx(h, j)
                    sco(h, j + 2)
                ctx(h, 6)
                ctx(h, 7)
            denom_pre(HPC - 1)
            bcast_norm(HPC - 1)

            # ---- partial output projection (tail) ----
            nd = HPC * HD // P  # 4 d-tiles
            for et in range(NT):
                ot = out_pool.tile([P, L], bf16, tag="osb")
                ps = psum.tile([P, L], f32, tag="sc", name=f"ops{et}")
                for u in range(2):
                    for dt in range(nd):
                        nc.tensor.matmul(
                            ps[:, 512 * u : 512 * (u + 1)],
                            wout_sb[:, dt, P * et : P * (et + 1)],
                            ctx_sb[:, dt, 512 * u : 512 * (u + 1)],
                            start=(dt == 0),
                            stop=(dt == nd - 1),
                        )
                # casts/DMAs alternate engines/queues so the tail drains fast
                if et % 2 == 0:
                    with nc.allow_low_precision(reason="bf16 partial out"):
                        nc.vector.tensor_copy(ot[:], ps[:])
                else:
                    nc.scalar.copy(ot[:], ps[:])
                (nc.gpsimd if et % 2 else nc.sync).dma_start(
                    outp_d[P * et : P * (et + 1), :], ot[:]
                )

    nc.compile()
    return nc


def _prep_core_inputs(x, mask, w_qkv, w_out, rel_pos_bias):
    """Host-side sharding/layout prep.  Returns in_maps for the 8 cores."""
    w3 = w_qkv.reshape(D, 3, H, HD)
    madd_t = np.where(mask[0, 0], np.float32(0), np.float32(NEG)).T  # [k, q]
    scale = np.float32(HD**-0.5)

    in_maps = []
    for c in range(8):
        b, g = divmod(c, 2)
        hs = slice(g * HPC, (g + 1) * HPC)
        # xt packed [p, u, dt, c]: SBUF layout, 4KB-contiguous DMA chunks
        xt = np.ascontiguousarray(
            x[b].T.reshape(NT, P, 2, 512).transpose(1, 2, 0, 3).reshape(P, -1)
        ).astype(BF16)
        qpart = (w3[:, 0, hs, :] * scale).reshape(D, HPC * HD)
        kpart = w3[:, 1, hs, :].reshape(D, HPC * HD)
        wqk_flat = np.concatenate([qpart, kpart], axis=1)  # [D, 1024]
        # packed [p, t, dt, c]: one contiguous 2KB/partition DMA per col-tile
        wqk = np.ascontiguousarray(
            wqk_flat.reshape(NT, P, NT, P).transpose(1, 2, 0, 3).reshape(P, -1)
        ).astype(BF16)
        wv = np.ascontiguousarray(
            w3[:, 2, hs, :].reshape(NT, P, HPC * HD).transpose(1, 0, 2).reshape(P, -1)
        ).astype(BF16)
        # w_out rows: per pair, second head first (matches ctx_sb layout
        # where the odd head writes partitions 0-63 directly)
        wo = w_out[g * HPC * HD : (g + 1) * HPC * HD, :]
        wo = wo.reshape(4, 2, HD, D)[:, ::-1].reshape(HPC * HD, D)
        wout = np.ascontiguousarray(
            wo.reshape(4, P, D).transpose(1, 0, 2).reshape(P, -1)
        ).astype(BF16)

        # multiplicative bias table: exp(bias + additive mask), head-major
        biasp = np.empty((P, _BIAS_COLS), dtype=np.float32)
        bt = rel_pos_bias[hs].transpose(0, 2, 1)  # [8, k, q]
        for hl in range(HPC):
            for j in range(NT):
                blk = bt[hl, P * j : P * (j + 1), P * j : L] + madd_t[
                    P * j : P * (j + 1), P * j : L
                ]  # [128, W_j]
                o = hl * _HEAD_COLS + _OFF_J[j]
                biasp[:, o : o + _W[j]] = np.exp(blk)
        in_maps.append(
            {
                "xt": xt,
                "wqk": wqk,
                "wv": wv,
                "wout": wout,
                "biasp": biasp.astype(BF16),
            }
        )
    return in_maps


# test-harness hooks (ignored in normal grading use)
PROFILE_DIR = None
TRACE_CORES = None
LAST_RESULT = None


def kernel(x, mask, w_qkv, w_out, rel_pos_bias):
    from concourse.bass_utils import run_bass_kernel_spmd

    global LAST_RESULT
    nc = _build()
    in_maps = _prep_core_inputs(x, mask, w_qkv, w_out, rel_pos_bias)
    kwargs = {}
    if PROFILE_DIR is not None:
        kwargs = dict(
            trace=True,
            tmpdir=PROFILE_DIR,
            trace_cores=TRACE_CORES,
        )
    res = run_bass_kernel_spmd(nc, in_maps, core_ids=list(range(8)), **kwargs)
    LAST_RESULT = res
    out = np.empty((B, L, D), np.float32)
    for b in range(B):
        part = res.results[2 * b]["outp"].astype(np.float32) + res.results[
            2 * b + 1
        ]["outp"].astype(np.float32)
        out[b] = part.T
    return out
